# revision 71
# baseline (speedup 1.0000x reference)
"""Trainium2 Bass kernel for nn_MultiHeadAttention_4964982194257.

Full-input contract: kernel(**inputs) takes the unsharded fp32 inputs and
returns the full [2, 2048, 2048] fp32 output.

Sharding (8 cores): data-parallel over batch (2) x tensor-parallel over
head groups (4).  Core c handles batch c//4 and q-heads [8g, 8g+8), g=c%4,
with the matching 2 GQA kv heads.  Each core computes a partial output
y_partial = attn_out_shard @ wo_shard; the host sums the 4 group partials
per batch.

On-core layout notes:
  * everything is computed in "feature-on-partition" transposed layouts:
    Q^T/K^T [f, t], scores S^T [tk, tq], attention out O^T [f, t],
    final y^T [o, t] (host transposes back).
  * q-head order within a core is permuted to [0,4,1,5,2,6,3,7] so that a
    128-row f-tile holds heads (i, i+4) whose kv heads are (kv0, kv1) --
    exactly matching K^T's natural partition layout.
  * softmax denominator comes from a ones-column appended to V in the
    A@V matmul (65th output row); division happens after a PE broadcast
    of the reciprocal row.

Scheduling notes (v2):
  * inputs land in few, large DMAs (HWDGE charges a fixed ~625ns per DMA)
    split across both hardware queues (SP + Activation), emitted in global
    need-order because the shared DMA engine pool serves transfers in
    acquisition order.
  * rope cos/sin tables ship compact ([32, SEQ]) and are expanded on-chip
    in place by a selector matmul, saving 768 KB off the critical input
    stream.
  * the prefix is just K chunk 0 + Q f-tile-0 chunk 0; K chunks 1-3 are
    projected inside attention block (0,0) as x streams in, V tiles lag
    one iteration behind, and every later block carries exactly one
    deferred Q-projection chunk so the PE stays ahead of the exp (Act)
    engine everywhere.
  * denominators are stored block-major (row 2i+u) and split into two
    tiles (pairs 0-2 / pair 3) so reciprocals for pairs 0-2 run one block
    early and the current chunk's pair 0-2 divisions happen inside block
    (3,j); only pair 3 of the last chunk remains in the tail.
  * o-projection copy-out goes through a 6-slot ring DMA'd in pairs; the
    tail borrows S-ring PSUM slots, copies on the (idle) Act engine, and
    eight deferred chunk-2 tiles cover the pair-3 recip latency chain.
"""

import math
import os
import sys

import numpy as np

for _p in ("/opt/trn_rl_repo", os.path.expanduser("~/.axon_site/_ro/trn_rl_repo")):
    if os.path.isdir(_p) and _p not in sys.path:
        sys.path.append(_p)

import ml_dtypes  # noqa: E402
import concourse.bass as bass  # noqa: E402
from concourse import bacc  # noqa: E402
import concourse.mybir as mybir  # noqa: E402
import concourse.tile as tile  # noqa: E402
from concourse import bass_utils  # noqa: E402

BF16NP = ml_dtypes.bfloat16

HIDDEN = 2048
NUM_HEADS = 32
NUM_KV_HEADS = 8
HEAD_DIM = 64
GROUPS = 4
SEQ = 2048
BATCH = 2
NCORES = 8
FH = 512  # features per core (8 q heads * 64)
PERM = [0, 4, 1, 5, 2, 6, 3, 7]  # local q-head order within a core

BF = mybir.dt.bfloat16
F32 = mybir.dt.float32

_CACHE = {}


# ----------------------------------------------------------------- host math
def _yarn_cos_sin():
    """Replicates reference._yarn_cos_sin for seq_len=SEQ. [SEQ, 32] f32."""
    dim = HEAD_DIM
    rope_base = 10000.0
    yarn_factor = 195.3
    max_seq = 4096
    pos_freqs = rope_base ** (np.arange(0, dim, 2, dtype=np.float64) / dim)
    inv_extra = 1.0 / pos_freqs
    inv_inter = 1.0 / (yarn_factor * pos_freqs)

    def corr_dim(num_rot):
        return (
            dim
            * math.log(max_seq / (num_rot * 2 * math.pi))
            / (2 * math.log(rope_base))
        )

    low = max(math.floor(corr_dim(32.0)), 0)
    high = min(math.ceil(corr_dim(1.0)), dim // 2 - 1)
    ramp = np.clip(
        (np.arange(dim // 2, dtype=np.float64) - low) / max(high - low, 1e-3), 0.0, 1.0
    )
    extrap = 1.0 - ramp
    inv_freq = inv_inter * (1.0 - extrap) + inv_extra * extrap
    t = np.arange(SEQ, dtype=np.float64)
    freqs = np.outer(t, inv_freq)
    mscale = 0.1 * math.log(yarn_factor) + 1.0
    cos = (np.cos(freqs) * mscale).astype(np.float32)
    sin = (np.sin(freqs) * mscale).astype(np.float32)
    return cos, sin


def _host_constants():
    cos, sin = _yarn_cos_sin()
    # compact rope tables [32, SEQ]; expanded on-chip to [128, SEQ]
    # (row r <- compact row (r%64)//2) via a selector matmul
    cosE = np.ascontiguousarray(cos.T).astype(BF16NP)  # [32, SEQ]
    sinE = np.ascontiguousarray(sin.T).astype(BF16NP)
    xpand = np.zeros((32, 128), dtype=BF16NP)
    for r in range(128):
        xpand[(r % 64) // 2, r] = 1.0

    # signed pair-swap permutation: rot = P.T @ q ; rot[2i] = -q[2i+1],
    # rot[2i+1] = q[2i]
    rotP = np.zeros((128, 128), dtype=BF16NP)
    for i in range(64):
        rotP[2 * i + 1, 2 * i] = -1.0
        rotP[2 * i, 2 * i + 1] = 1.0

    # row-selector for reciprocal broadcast.  Denominators are stored
    # block-major (row = 2*i + u for head h = i + 4u) so rows 0:6 (pairs
    # 0-2) can be gathered and reciprocal'd one attention block earlier
    # than rows 6:8 (pair 3).
    sel8 = np.zeros((8, 512), dtype=np.float32)
    for h in range(8):
        r = 2 * (h % 4) + h // 4
        sel8[r, h * 64 : (h + 1) * 64] = 1.0
    return cosE, sinE, rotP, sel8, xpand


def _split_pk(ap, p=128):
    """[(k p), c] DRAM slice -> [p, k, c] access pattern."""
    return ap.rearrange("(k p) c -> p k c", p=p)


# --------------------------------------------------------------- bass kernel
def _emit(tc, nc, aps):
    P = 128
    Exp = mybir.ActivationFunctionType.Exp
    Copy = mybir.ActivationFunctionType.Copy
    mult = mybir.AluOpType.mult
    addop = mybir.AluOpType.add

    xT, wqT, wkT, wvT, woT, cosD, sinD, rotD, selD, xpD, yT = aps

    cst = tc.alloc_tile_pool(name="cst", bufs=1)
    big = tc.alloc_tile_pool(name="big", bufs=1)
    wts = tc.alloc_tile_pool(name="wts", bufs=1)
    tmp = tc.alloc_tile_pool(name="tmp", bufs=2)
    dram = tc.alloc_tile_pool(name="dram", bufs=2, space="DRAM")
    # single PSUM pool, 8 banks: S x2 (2 banks each), pav x2, scr x2.
    pp = tc.alloc_tile_pool(name="pp", bufs=1, space="PSUM")

    def S_tile():
        return pp.tile([P, 1024], F32, tag="S", bufs=2, name="s_ps")

    def pav_tile():
        return pp.tile([P, 512], F32, tag="pav", bufs=2, name="pav_ps")

    def scr_tile():
        return pp.tile([P, 512], F32, tag="scr", bufs=2, name="scr_ps")

    # ---- constants and inputs
    cos_sb = cst.tile([P, SEQ], BF)
    sin_sb = cst.tile([P, SEQ], BF)
    rot_sb = cst.tile([P, P], BF)
    sel_sb = cst.tile([8, 512], F32)

    xT_sb = big.tile([P, 16, SEQ], BF)
    wq_sb = wts.tile([P, 16, FH], BF)
    wk_sb = wts.tile([P, 16, 128], BF)
    wv_sb = wts.tile([P, 16, 128], BF)
    wo_sb = big.tile([P, 4, SEQ], BF)

    # Input DMAs in global need-order: the shared DMA engine pool serves
    # transfers in acquisition (= emission) order, so this order IS the
    # bandwidth priority.  x + wo ride the SP queue, weights the Act queue
    # (splitting the fixed ~625ns/DMA HWDGE cost across both queues).
    selB_sb = cst.tile([2, 512], F32)

    def x_half(j, k0, n):
        jc = slice(j * 512, (j + 1) * 512)
        if n == 1:
            nc.sync.dma_start(xT_sb[:, k0, jc], xT[k0 * P : (k0 + 1) * P, jc])
        else:
            nc.sync.dma_start(
                xT_sb[:, k0 : k0 + n, jc],
                _split_pk(xT[k0 * P : (k0 + n) * P, jc]),
            )

    xp_sb = cst.tile([32, 128], BF)
    nc.scalar.dma_start(wk_sb[:, 0:4, :], _split_pk(wkT[0 : 4 * P, :]))
    x_half(0, 0, 4)
    nc.scalar.dma_start(wk_sb[:, 4:16, :], _split_pk(wkT[4 * P :, :]))
    x_half(0, 4, 4)
    x_half(0, 8, 8)
    nc.scalar.dma_start(rot_sb, rotD)
    nc.scalar.dma_start(xp_sb, xpD)
    nc.scalar.dma_start(cos_sb[0:32, :], cosD)  # compact; expanded on-chip
    nc.scalar.dma_start(sin_sb[0:32, :], sinD)
    nc.scalar.dma_start(wq_sb[:, :, 0:128], _split_pk(wqT[:, 0:128]))
    nc.scalar.dma_start(wv_sb, _split_pk(wvT[:, :]))
    for j in range(1, 4):
        x_half(j, 0, 8)
        x_half(j, 8, 8)
    nc.scalar.dma_start(wq_sb[:, :, 128:256], _split_pk(wqT[:, 128:256]))
    nc.scalar.dma_start(wq_sb[:, :, 256:384], _split_pk(wqT[:, 256:384]))
    nc.scalar.dma_start(sel_sb, selD)
    nc.scalar.dma_start(selB_sb, selD[6:8, :])
    nc.scalar.dma_start(wq_sb[:, :, 384:512], _split_pk(wqT[:, 384:512]))
    nc.sync.dma_start(wo_sb, _split_pk(woT[:, :]))

    Qr_sb = big.tile([P, 4, SEQ], BF)  # rope'd Q^T, f-tile i = heads (i, i+4)
    Kr_sb = big.tile([P, SEQ], BF)  # rope'd K^T (kv0 rows 0:64, kv1 64:128)
    V_sb = big.tile([P, 16, 130], BF)  # [t-tile][kv0 64 | 1 | kv1 64 | 1]
    E_sb = big.tile([P, 5, 2, 512], BF)  # exp(S^T) ring buffer over tk tiles
    OT_sb = big.tile([P, 4, SEQ], BF)  # normalized attn out, feature layout
    Oraw = big.tile([64, 8, 512], BF)
    # denominator/reciprocal, split so pairs 0-2 (rows 0:6) decouple from
    # pair 3 (rows 6:8); row = 2*i + u for head h = i + 4u
    denA = big.tile([6, 512], F32)
    denB = big.tile([2, 512], F32)
    recA = big.tile([6, 512], F32)
    recB = big.tile([2, 512], F32)
    rscr = big.tile([6, 512], F32)
    ysb4 = big.tile([P, 6, 512], F32)  # o-proj copy-out ring, DMA'd in pairs

    def rope_chunk(dst, src_ps, j, rps_alloc=None):
        jc = slice(j * 512, (j + 1) * 512)
        qtmp = tmp.tile([P, 512], BF, tag="qtmp", bufs=2, name="qtmp")
        nc.vector.tensor_copy(qtmp, src_ps[:, 0:512])
        rps = (rps_alloc or pav_tile)()
        nc.tensor.matmul(rps, rot_sb, qtmp, start=True, stop=True)
        m1 = tmp.tile([P, 512], BF, tag="m1", bufs=1, name="m1")
        nc.vector.tensor_tensor(m1, qtmp, cos_sb[:, jc], op=mult)
        m2 = tmp.tile([P, 512], BF, tag="m2", bufs=1, name="m2")
        nc.vector.tensor_tensor(m2, rps, sin_sb[:, jc], op=mult)
        nc.vector.tensor_tensor(dst, m1, m2, op=addop)

    # ---- serial prefix: only K chunk 0 + Q f-tile 0 chunk 0.  K chunks 1-3
    # are emitted inside attention block (0,0) as x streams in; Q f-tile 0
    # chunks 1-3 are deferred fillers like the other f-tiles.
    def kproj_chunk(c):
        ps = scr_tile()
        for k in range(16):
            nc.tensor.matmul(
                ps,
                wk_sb[:, k, :],
                xT_sb[:, k, c * 512 : (c + 1) * 512],
                start=(k == 0),
                stop=(k == 15),
            )
        rope_chunk(Kr_sb[:, c * 512 : (c + 1) * 512], ps, c, rps_alloc=scr_tile)

    def expand_rope(c):
        # cos/sin row r = compact row (r%64)//2, via selector matmul; the
        # in-place overwrite of rows 0:32 is safe (per-column independence)
        cc = slice(c * 512, (c + 1) * 512)
        for tbl, act_copy in ((cos_sb, True), (sin_sb, False)):
            eps = scr_tile()
            nc.tensor.matmul(eps, xp_sb, tbl[0:32, cc], start=True, stop=True)
            if act_copy:
                nc.scalar.activation(tbl[:, cc], eps, Copy)  # Act idle here
            else:
                nc.vector.tensor_copy(tbl[:, cc], eps)

    ps = S_tile()
    for k in range(16):
        nc.tensor.matmul(
            ps[:, 0:512], wk_sb[:, k, :], xT_sb[:, k, 0:512],
            start=(k == 0), stop=(k == 15),
        )
    expand_rope(0)
    ps2 = S_tile()
    for k in range(8):
        nc.tensor.matmul(
            ps2[:, 0:512], wq_sb[:, k, 0:128], xT_sb[:, k, 0:512],
            start=(k == 0), stop=False,
        )
    rope_chunk(Kr_sb[:, 0:512], ps, 0)
    for k in range(8, 16):
        nc.tensor.matmul(
            ps2[:, 0:512], wq_sb[:, k, 0:128], xT_sb[:, k, 0:512],
            start=False, stop=(k == 15),
        )
    rope_chunk(Qr_sb[:, 0, 0:512], ps2, 0)
    expand_rope(1)
    expand_rope(2)
    expand_rope(3)

    nc.vector.memset(V_sb, 1.0)

    def v_chunk(t):
        # one V-projection tile, emitted inside the (0,0) attention iteration
        vps = scr_tile()
        for k in range(16):
            nc.tensor.matmul(
                vps[:, 0:128],
                xT_sb[:, k, t * P : (t + 1) * P],
                wv_sb[:, k, :],
                start=(k == 0),
                stop=(k == 15),
            )
        nc.vector.tensor_copy(V_sb[:, t, 0:64], vps[:, 0:64])
        nc.vector.tensor_copy(V_sb[:, t, 65:129], vps[:, 64:128])

    def q_chunk(fi, j):
        # one deferred Q-projection chunk, emitted inside an attention block
        ps = scr_tile()
        for k in range(16):
            nc.tensor.matmul(
                ps,
                wq_sb[:, k, fi * P : (fi + 1) * P],
                xT_sb[:, k, j * 512 : (j + 1) * 512],
                start=(k == 0),
                stop=(k == 15),
            )
        rope_chunk(Qr_sb[:, fi, j * 512 : (j + 1) * 512], ps, j)

    # ---- phase 2: attention + output projection, per 512-token q chunk
    def oproj_tile(j, m, tail=False, single_dma=False):
        jc = slice(j * 512, (j + 1) * 512)
        if tail and m % 2 == 1:
            yps = S_tile()[:, 0:512]  # borrow an S-ring slot: deeper ring
        else:
            yps = scr_tile()
        for k2 in range(4):
            nc.tensor.matmul(
                yps,
                wo_sb[:, k2, m * P : (m + 1) * P],
                OT_sb[:, k2, jc],
                start=(k2 == 0),
                stop=(k2 == 3),
            )
        if tail:
            nc.scalar.activation(ysb4[:, m % 6, :], yps, Copy)  # Act idle here
        else:
            nc.vector.tensor_copy(ysb4[:, m % 6, :], yps)
        if single_dma:
            # last tiles: don't batch, so the final transfer is short
            s0 = m % 6
            nc.sync.dma_start(
                yT[m * P : (m + 1) * P, jc], ysb4[:, s0, :]
            )
        elif m % 2 == 1:
            # one DMA per pair of m-tiles (HWDGE charges ~625ns per DMA)
            s0 = (m - 1) % 6
            nc.sync.dma_start(
                _split_pk(yT[(m - 1) * P : (m + 1) * P, jc]),
                ysb4[:, s0 : s0 + 2, :],
            )

    def emit_divisions(jd, pairs, tail=False):
        # normalize head pair i2 (heads i2, i2+4) of chunk jd.  Pairs 0-2
        # read recA (ready one block early); pair 3 reads recB.
        if jd < 0:
            return
        jcd = slice(jd * 512, (jd + 1) * 512)
        for i2 in pairs:
            for u in (0, 1):
                h = i2 + 4 * u
                rec = recB if i2 == 3 else recA
                rsel = selB_sb if i2 == 3 else sel_sb[0:6, :]
                rps = scr_tile()
                nc.tensor.matmul(
                    rps[0:64, :],
                    rsel[:, h * 64 : (h + 1) * 64],
                    rec,
                    start=True,
                    stop=True,
                )
                if u == 0:
                    nc.vector.tensor_tensor(
                        OT_sb[0:64, i2, jcd], Oraw[:, h, :], rps[0:64, :], op=mult
                    )
                else:
                    otmp = tmp.tile([64, 512], BF, tag="otmp", bufs=2, name="otmp")
                    nc.vector.tensor_tensor(
                        otmp, Oraw[:, h, :], rps[0:64, :], op=mult
                    )
                    dma_eng = nc.scalar if tail else nc.sync
                    dma_eng.dma_start(OT_sb[64:128, i2, jcd], otmp)

    for j in range(4):
        jc = slice(j * 512, (j + 1) * 512)
        den_dram = dram.tile([8, 512], F32, tag="dend", bufs=2, name="dend")
        for i in range(4):
            pavA = pav_tile()
            pavB = pav_tile()

            def av_step(k):
                for u, pav in ((0, pavA), (1, pavB)):
                    nc.tensor.matmul(
                        pav[0:65, :],
                        V_sb[:, k, u * 65 : (u + 1) * 65],
                        E_sb[:, k % 5, u, :],
                        start=(k == 0),
                        stop=(k == 15),
                    )

            for k in range(16):
                S_t = S_tile()
                ks = slice(k * P, (k + 1) * P)
                nc.tensor.matmul(
                    S_t[:, 0:512], Kr_sb[0:64, ks], Qr_sb[0:64, i, jc],
                    start=True, stop=True,
                )
                nc.tensor.matmul(
                    S_t[:, 512:1024], Kr_sb[64:128, ks], Qr_sb[64:128, i, jc],
                    start=True, stop=True,
                )
                nc.scalar.activation(
                    E_sb[:, k % 5, :, :],
                    S_t.rearrange("p (u c) -> p u c", c=512),
                    Exp,
                    scale=0.125,
                )
                # fill PE slack: V tiles + remaining K chunks during (0,0),
                # both lagged to match the x DMA stream; exactly one deferred
                # Q chunk per block; previous chunk's o-proj during j>0;
                # current chunk's pair 0-2 divisions at i==3
                if j == 0 and i == 0:
                    if k >= 1:
                        v_chunk(k - 1)
                    if k in (3, 7, 11):
                        kproj_chunk((k + 1) // 4)
                if k == (14 if (j, i) == (0, 0) else 1):
                    fi, jq = (i + 1, j) if i < 3 else (0, j + 1)
                    if jq < 4 and not (fi == 0 and jq == 0):
                        q_chunk(fi, jq)
                if k >= 2:
                    av_step(k - 2)
                oproj_ks = (8, 9, 13, 15) if i == 0 else (7, 9, 13, 15)
                if j > 0 and k in oproj_ks:
                    if not (j == 3 and k >= 11):
                        oproj_tile(j - 1, i * 4 + oproj_ks.index(k))
                if i == 0 and k == 4:
                    emit_divisions(j - 1, [3])  # leftover pair of prev chunk
                if i == 3 and k in (5, 10, 14):
                    emit_divisions(j, [(k == 10) + 2 * (k == 14)])
            if (j, i) == (0, 0):
                v_chunk(15)
            av_step(14)
            av_step(15)
            # denominator copies + DMAs first (they head the recip chain the
            # next divisions wait on), Oraw copies after (consumed later)
            dens = []
            for u, pav in ((0, pavA), (1, pavB)):
                den1 = tmp.tile([P, 512], F32, tag="den1", bufs=2, name="den1")
                nc.vector.tensor_copy(den1[64:65, :], pav[64:65, :])
                dens.append(den1)
            for u, den1 in enumerate(dens):
                nc.sync.dma_start(den_dram[2 * i + u : 2 * i + u + 1, :],
                                  den1[64:65, :])
            for u, pav in ((0, pavA), (1, pavB)):
                nc.vector.tensor_copy(Oraw[:, i + 4 * u, :], pav[0:64, :])
            if i == 2:
                # pairs 0-2 denominators complete: reciprocal one block early
                nc.sync.dma_start(denA, den_dram[0:6, :])
                nc.vector.reciprocal_approx_accurate(recA, denA, scratch=rscr)
        nc.sync.dma_start(denB, den_dram[6:8, :])
        nc.vector.reciprocal_approx_accurate(recB, denB, scratch=rscr[0:2, :])

    # tail: chunk 2's eight deferred o-proj tiles overlap the pair-3 recip
    # chain, then pair 3 of chunk 3 divides, then the last chunk's o-proj
    # (k2 order puts pair 3 last, so only m=0 briefly waits)
    for m in (2, 3, 6, 7, 10, 11, 14, 15):
        oproj_tile(2, m, tail=True)
    emit_divisions(3, [3], tail=True)
    for m in range(16):
        oproj_tile(3, m, tail=True, single_dma=(m >= 14))

    for p in (pp, dram, tmp, wts, big, cst):
        p.release()


def _build():
    if "nc" in _CACHE:
        return _CACHE["nc"]
    nc = bacc.Bacc("TRN2", target_bir_lowering=False, debug=False, num_devices=NCORES)
    xT = nc.dram_tensor("xT", [HIDDEN, SEQ], BF, kind="ExternalInput").ap()
    wqT = nc.dram_tensor("wqT", [HIDDEN, FH], BF, kind="ExternalInput").ap()
    wkT = nc.dram_tensor("wkT", [HIDDEN, 128], BF, kind="ExternalInput").ap()
    wvT = nc.dram_tensor("wvT", [HIDDEN, 128], BF, kind="ExternalInput").ap()
    woT = nc.dram_tensor("woT", [FH, HIDDEN], BF, kind="ExternalInput").ap()
    cosD = nc.dram_tensor("cosE", [32, SEQ], BF, kind="ExternalInput").ap()
    sinD = nc.dram_tensor("sinE", [32, SEQ], BF, kind="ExternalInput").ap()
    xpD = nc.dram_tensor("xpand", [32, 128], BF, kind="ExternalInput").ap()
    rotD = nc.dram_tensor("rotP", [128, 128], BF, kind="ExternalInput").ap()
    selD = nc.dram_tensor("sel8", [8, 512], F32, kind="ExternalInput").ap()
    yT = nc.dram_tensor("yT", [HIDDEN, SEQ], F32, kind="ExternalOutput").ap()
    with tile.TileContext(nc) as tc:
        _emit(tc, nc, (xT, wqT, wkT, wvT, woT, cosD, sinD, rotD, selD, xpD, yT))
    nc.compile()
    _CACHE["nc"] = nc
    return nc


def _in_maps(hidden_states, wq, wk, wv, wo):
    cosE, sinE, rotP, sel8, xpand = _host_constants()
    maps = []
    for c in range(NCORES):
        b, g = c // 4, c % 4
        feat = np.concatenate(
            [np.arange(64) + 64 * (8 * g + hl) for hl in PERM]
        )
        maps.append(
            {
                "xT": np.ascontiguousarray(hidden_states[b].T).astype(BF16NP),
                "wqT": np.ascontiguousarray(wq[feat, :].T).astype(BF16NP),
                "wkT": np.ascontiguousarray(
                    wk[128 * g : 128 * (g + 1), :].T
                ).astype(BF16NP),
                "wvT": np.ascontiguousarray(
                    wv[128 * g : 128 * (g + 1), :].T
                ).astype(BF16NP),
                "woT": np.ascontiguousarray(wo[:, feat].T).astype(BF16NP),
                "cosE": cosE,
                "sinE": sinE,
                "xpand": xpand,
                "rotP": rotP,
                "sel8": sel8,
            }
        )
    return maps


def kernel(hidden_states, wq, wk, wv, wo):
    nc = _build()
    maps = _in_maps(
        np.asarray(hidden_states, dtype=np.float32),
        np.asarray(wq, dtype=np.float32),
        np.asarray(wk, dtype=np.float32),
        np.asarray(wv, dtype=np.float32),
        np.asarray(wo, dtype=np.float32),
    )
    res = bass_utils.run_bass_kernel_spmd(nc, maps, list(range(NCORES))).results
    y = np.zeros((BATCH, SEQ, HIDDEN), dtype=np.float64)
    for c in range(NCORES):
        y[c // 4] += res[c]["yT"].T.astype(np.float64)
    return y.astype(np.float32)


# revision 78
# speedup vs baseline: 3.9904x; 3.9904x over previous
"""Trainium2 Bass kernel for nn_MultiHeadAttention_4964982194257.

Full-input contract: kernel(**inputs) takes the unsharded fp32 inputs and
returns the full [2, 2048, 2048] fp32 output.

Sharding (8 cores): data-parallel over batch (2) x tensor-parallel over
head groups (4).  Core c handles batch c//4 and q-heads [8g, 8g+8), g=c%4,
with the matching 2 GQA kv heads.  Each core computes a partial output
y_partial = attn_out_shard @ wo_shard; the host sums the 4 group partials
per batch.

On-core layout notes:
  * everything is computed in "feature-on-partition" transposed layouts:
    Q^T/K^T [f, t], scores S^T [tk, tq], attention out O^T [f, t],
    final y^T [o, t] (host transposes back).
  * q-head order within a core is permuted to [0,4,1,5,2,6,3,7] so that a
    128-row f-tile holds heads (i, i+4) whose kv heads are (kv0, kv1) --
    exactly matching K^T's natural partition layout.
  * softmax denominator comes from a ones-column appended to V in the
    A@V matmul (65th output row); division happens after a PE broadcast
    of the reciprocal row.

Scheduling notes (v2):
  * inputs land in few, large DMAs (HWDGE charges a fixed ~625ns per DMA)
    split across both hardware queues (SP + Activation), emitted in global
    need-order because the shared DMA engine pool serves transfers in
    acquisition order.
  * rope cos/sin tables ship compact ([32, SEQ]) and are expanded on-chip
    in place by a selector matmul, saving 768 KB off the critical input
    stream.
  * the prefix is just K chunk 0 + Q f-tile-0 chunk 0; K chunks 1-3 are
    projected inside attention block (0,0) as x streams in, V tiles lag
    one iteration behind, and every later block carries exactly one
    deferred Q-projection chunk so the PE stays ahead of the exp (Act)
    engine everywhere.
  * denominators are stored block-major (row 2i+u) and split into two
    tiles (pairs 0-2 / pair 3) so reciprocals for pairs 0-2 run one block
    early and the current chunk's pair 0-2 divisions happen inside block
    (3,j); only pair 3 of the last chunk remains in the tail.
  * o-projection copy-out goes through a 6-slot ring DMA'd in pairs; the
    tail borrows S-ring PSUM slots, copies on the (idle) Act engine, and
    eight deferred chunk-2 tiles cover the pair-3 recip latency chain.
"""

import math
import os
import sys

import numpy as np

for _p in ("/opt/trn_rl_repo", os.path.expanduser("~/.axon_site/_ro/trn_rl_repo")):
    if os.path.isdir(_p) and _p not in sys.path:
        sys.path.append(_p)

import ml_dtypes  # noqa: E402
import concourse.bass as bass  # noqa: E402
from concourse import bacc  # noqa: E402
import concourse.mybir as mybir  # noqa: E402
import concourse.tile as tile  # noqa: E402
from concourse import bass_utils  # noqa: E402

BF16NP = ml_dtypes.bfloat16

HIDDEN = 2048
NUM_HEADS = 32
NUM_KV_HEADS = 8
HEAD_DIM = 64
GROUPS = 4
SEQ = 2048
BATCH = 2
NCORES = 8
FH = 512  # features per core (8 q heads * 64)
PERM = [0, 4, 1, 5, 2, 6, 3, 7]  # local q-head order within a core

BF = mybir.dt.bfloat16
F32 = mybir.dt.float32

_CACHE = {}


# ----------------------------------------------------------------- host math
def _yarn_cos_sin():
    """Replicates reference._yarn_cos_sin for seq_len=SEQ. [SEQ, 32] f32."""
    dim = HEAD_DIM
    rope_base = 10000.0
    yarn_factor = 195.3
    max_seq = 4096
    pos_freqs = rope_base ** (np.arange(0, dim, 2, dtype=np.float64) / dim)
    inv_extra = 1.0 / pos_freqs
    inv_inter = 1.0 / (yarn_factor * pos_freqs)

    def corr_dim(num_rot):
        return (
            dim
            * math.log(max_seq / (num_rot * 2 * math.pi))
            / (2 * math.log(rope_base))
        )

    low = max(math.floor(corr_dim(32.0)), 0)
    high = min(math.ceil(corr_dim(1.0)), dim // 2 - 1)
    ramp = np.clip(
        (np.arange(dim // 2, dtype=np.float64) - low) / max(high - low, 1e-3), 0.0, 1.0
    )
    extrap = 1.0 - ramp
    inv_freq = inv_inter * (1.0 - extrap) + inv_extra * extrap
    t = np.arange(SEQ, dtype=np.float64)
    freqs = np.outer(t, inv_freq)
    mscale = 0.1 * math.log(yarn_factor) + 1.0
    cos = (np.cos(freqs) * mscale).astype(np.float32)
    sin = (np.sin(freqs) * mscale).astype(np.float32)
    return cos, sin


def _host_constants():
    cos, sin = _yarn_cos_sin()
    # compact rope tables [32, SEQ]; expanded on-chip to [128, SEQ]
    # (row r <- compact row (r%64)//2) via a selector matmul
    cosE = np.ascontiguousarray(cos.T).astype(BF16NP)  # [32, SEQ]
    sinE = np.ascontiguousarray(sin.T).astype(BF16NP)
    xpand = np.zeros((32, 128), dtype=BF16NP)
    for r in range(128):
        xpand[(r % 64) // 2, r] = 1.0

    # signed pair-swap permutation: rot = P.T @ q ; rot[2i] = -q[2i+1],
    # rot[2i+1] = q[2i]
    rotP = np.zeros((128, 128), dtype=BF16NP)
    for i in range(64):
        rotP[2 * i + 1, 2 * i] = -1.0
        rotP[2 * i, 2 * i + 1] = 1.0

    # row-selector for reciprocal broadcast.  Denominators are stored
    # block-major (row = 2*i + u for head h = i + 4u) so rows 0:6 (pairs
    # 0-2) can be gathered and reciprocal'd one attention block earlier
    # than rows 6:8 (pair 3).
    sel8 = np.zeros((8, 512), dtype=np.float32)
    for h in range(8):
        r = 2 * (h % 4) + h // 4
        sel8[r, h * 64 : (h + 1) * 64] = 1.0
    return cosE, sinE, rotP, sel8, xpand


def _split_pk(ap, p=128):
    """[(k p), c] DRAM slice -> [p, k, c] access pattern."""
    return ap.rearrange("(k p) c -> p k c", p=p)


# --------------------------------------------------------------- bass kernel
def _emit(tc, nc, aps):
    P = 128
    Exp = mybir.ActivationFunctionType.Exp
    Copy = mybir.ActivationFunctionType.Copy
    mult = mybir.AluOpType.mult
    addop = mybir.AluOpType.add

    xT, wqT, wkT, wvT, woT, cosD, sinD, rotD, selD, xpD, yT = aps

    cst = tc.alloc_tile_pool(name="cst", bufs=1)
    big = tc.alloc_tile_pool(name="big", bufs=1)
    wts = tc.alloc_tile_pool(name="wts", bufs=1)
    tmp = tc.alloc_tile_pool(name="tmp", bufs=2)
    dram = tc.alloc_tile_pool(name="dram", bufs=2, space="DRAM")
    # single PSUM pool, 8 banks: S x2 (2 banks each), pav x2, scr x2.
    pp = tc.alloc_tile_pool(name="pp", bufs=1, space="PSUM")

    def S_tile():
        return pp.tile([P, 1024], F32, tag="S", bufs=2, name="s_ps")

    def pav_tile():
        return pp.tile([P, 512], F32, tag="pav", bufs=2, name="pav_ps")

    def scr_tile():
        return pp.tile([P, 512], F32, tag="scr", bufs=2, name="scr_ps")

    # ---- constants and inputs
    cos_sb = cst.tile([P, SEQ], BF)
    sin_sb = cst.tile([P, SEQ], BF)
    rot_sb = cst.tile([P, P], BF)
    sel_sb = cst.tile([8, 512], F32)

    xT_sb = big.tile([P, 16, SEQ], BF)
    wq_sb = wts.tile([P, 16, FH], BF)
    wk_sb = wts.tile([P, 16, 128], BF)
    wv_sb = wts.tile([P, 16, 128], BF)
    wo_sb = big.tile([P, 4, SEQ], BF)

    # Input DMAs in global need-order: the shared DMA engine pool serves
    # transfers in acquisition (= emission) order, so this order IS the
    # bandwidth priority.  x + wo ride the SP queue, weights the Act queue
    # (splitting the fixed ~625ns/DMA HWDGE cost across both queues).
    selB_sb = cst.tile([2, 512], F32)

    def x_half(j, k0, n):
        jc = slice(j * 512, (j + 1) * 512)
        if n == 1:
            nc.sync.dma_start(xT_sb[:, k0, jc], xT[k0 * P : (k0 + 1) * P, jc])
        else:
            nc.sync.dma_start(
                xT_sb[:, k0 : k0 + n, jc],
                _split_pk(xT[k0 * P : (k0 + n) * P, jc]),
            )

    xp_sb = cst.tile([32, 128], BF)
    nc.scalar.dma_start(wk_sb[:, 0:4, :], _split_pk(wkT[0 : 4 * P, :]))
    x_half(0, 0, 4)
    nc.scalar.dma_start(wk_sb[:, 4:16, :], _split_pk(wkT[4 * P :, :]))
    x_half(0, 4, 4)
    x_half(0, 8, 8)
    nc.scalar.dma_start(rot_sb, rotD)
    nc.scalar.dma_start(xp_sb, xpD)
    nc.scalar.dma_start(cos_sb[0:32, :], cosD)  # compact; expanded on-chip
    nc.scalar.dma_start(sin_sb[0:32, :], sinD)
    nc.scalar.dma_start(wq_sb[:, :, 0:128], _split_pk(wqT[:, 0:128]))
    nc.scalar.dma_start(wv_sb, _split_pk(wvT[:, :]))
    for j in range(1, 4):
        x_half(j, 0, 8)
        x_half(j, 8, 8)
    nc.scalar.dma_start(wq_sb[:, :, 128:256], _split_pk(wqT[:, 128:256]))
    nc.scalar.dma_start(wq_sb[:, :, 256:384], _split_pk(wqT[:, 256:384]))
    nc.scalar.dma_start(sel_sb, selD)
    nc.scalar.dma_start(selB_sb, selD[6:8, :])
    nc.scalar.dma_start(wq_sb[:, :, 384:512], _split_pk(wqT[:, 384:512]))
    nc.sync.dma_start(wo_sb, _split_pk(woT[:, :]))

    Qr_sb = big.tile([P, 4, SEQ], BF)  # rope'd Q^T, f-tile i = heads (i, i+4)
    Kr_sb = big.tile([P, SEQ], BF)  # rope'd K^T (kv0 rows 0:64, kv1 64:128)
    V_sb = big.tile([P, 16, 130], BF)  # [t-tile][kv0 64 | 1 | kv1 64 | 1]
    E_sb = big.tile([P, 5, 2, 512], BF)  # exp(S^T) ring buffer over tk tiles
    OT_sb = big.tile([P, 4, SEQ], BF)  # normalized attn out, feature layout
    Oraw = big.tile([64, 8, 512], BF)
    # denominator/reciprocal, split so pairs 0-2 (rows 0:6) decouple from
    # pair 3 (rows 6:8); row = 2*i + u for head h = i + 4u
    denA = big.tile([6, 512], F32)
    denB = big.tile([2, 512], F32)
    recA = big.tile([6, 512], F32)
    recB = big.tile([2, 512], F32)
    rscr = big.tile([6, 512], F32)
    ysb4 = big.tile([P, 6, 512], F32)  # o-proj copy-out ring, DMA'd in pairs

    def rope_chunk(dst, src_ps, j, rps_alloc=None):
        jc = slice(j * 512, (j + 1) * 512)
        qtmp = tmp.tile([P, 512], BF, tag="qtmp", bufs=2, name="qtmp")
        nc.vector.tensor_copy(qtmp, src_ps[:, 0:512])
        rps = (rps_alloc or pav_tile)()
        nc.tensor.matmul(rps, rot_sb, qtmp, start=True, stop=True)
        m1 = tmp.tile([P, 512], BF, tag="m1", bufs=1, name="m1")
        nc.vector.tensor_tensor(m1, qtmp, cos_sb[:, jc], op=mult)
        m2 = tmp.tile([P, 512], BF, tag="m2", bufs=1, name="m2")
        nc.vector.tensor_tensor(m2, rps, sin_sb[:, jc], op=mult)
        nc.vector.tensor_tensor(dst, m1, m2, op=addop)

    # ---- serial prefix: only K chunk 0 + Q f-tile 0 chunk 0.  K chunks 1-3
    # are emitted inside attention block (0,0) as x streams in; Q f-tile 0
    # chunks 1-3 are deferred fillers like the other f-tiles.
    def kproj_chunk(c):
        ps = scr_tile()
        for k in range(16):
            nc.tensor.matmul(
                ps,
                wk_sb[:, k, :],
                xT_sb[:, k, c * 512 : (c + 1) * 512],
                start=(k == 0),
                stop=(k == 15),
            )
        rope_chunk(Kr_sb[:, c * 512 : (c + 1) * 512], ps, c, rps_alloc=scr_tile)

    def expand_rope(c):
        # cos/sin row r = compact row (r%64)//2, via selector matmul; the
        # in-place overwrite of rows 0:32 is safe (per-column independence)
        cc = slice(c * 512, (c + 1) * 512)
        for tbl, act_copy in ((cos_sb, True), (sin_sb, False)):
            eps = scr_tile()
            nc.tensor.matmul(eps, xp_sb, tbl[0:32, cc], start=True, stop=True)
            if act_copy:
                nc.scalar.activation(tbl[:, cc], eps, Copy)  # Act idle here
            else:
                nc.vector.tensor_copy(tbl[:, cc], eps)

    ps = S_tile()
    for k in range(16):
        nc.tensor.matmul(
            ps[:, 0:512], wk_sb[:, k, :], xT_sb[:, k, 0:512],
            start=(k == 0), stop=(k == 15),
        )
    expand_rope(0)
    ps2 = S_tile()
    for k in range(8):
        nc.tensor.matmul(
            ps2[:, 0:512], wq_sb[:, k, 0:128], xT_sb[:, k, 0:512],
            start=(k == 0), stop=False,
        )
    rope_chunk(Kr_sb[:, 0:512], ps, 0)
    for k in range(8, 16):
        nc.tensor.matmul(
            ps2[:, 0:512], wq_sb[:, k, 0:128], xT_sb[:, k, 0:512],
            start=False, stop=(k == 15),
        )
    rope_chunk(Qr_sb[:, 0, 0:512], ps2, 0)

    nc.vector.memset(V_sb, 1.0)

    def v_chunk(t):
        # one V-projection tile, emitted inside the (0,0) attention iteration
        vps = scr_tile()
        for k in range(16):
            nc.tensor.matmul(
                vps[:, 0:128],
                xT_sb[:, k, t * P : (t + 1) * P],
                wv_sb[:, k, :],
                start=(k == 0),
                stop=(k == 15),
            )
        nc.vector.tensor_copy(V_sb[:, t, 0:64], vps[:, 0:64])
        nc.vector.tensor_copy(V_sb[:, t, 65:129], vps[:, 64:128])

    def q_chunk(fi, j):
        # one deferred Q-projection chunk, emitted inside an attention block
        ps = scr_tile()
        for k in range(16):
            nc.tensor.matmul(
                ps,
                wq_sb[:, k, fi * P : (fi + 1) * P],
                xT_sb[:, k, j * 512 : (j + 1) * 512],
                start=(k == 0),
                stop=(k == 15),
            )
        rope_chunk(Qr_sb[:, fi, j * 512 : (j + 1) * 512], ps, j)

    # ---- phase 2: attention + output projection, per 512-token q chunk
    def oproj_tile(j, m, tail=False, single_dma=False):
        jc = slice(j * 512, (j + 1) * 512)
        if tail and m % 2 == 1:
            yps = S_tile()[:, 0:512]  # borrow an S-ring slot: deeper ring
        else:
            yps = scr_tile()
        for k2 in range(4):
            nc.tensor.matmul(
                yps,
                wo_sb[:, k2, m * P : (m + 1) * P],
                OT_sb[:, k2, jc],
                start=(k2 == 0),
                stop=(k2 == 3),
            )
        if tail and m % 2 == 0:
            nc.scalar.activation(ysb4[:, m % 6, :], yps, Copy)  # Act idle here
        else:
            nc.vector.tensor_copy(ysb4[:, m % 6, :], yps)
        dma_eng = nc.sync
        if single_dma:
            # last tiles: don't batch, so the final transfer is short
            s0 = m % 6
            dma_eng.dma_start(
                yT[m * P : (m + 1) * P, jc], ysb4[:, s0, :]
            )
        elif m % 2 == 1:
            # one DMA per pair of m-tiles (HWDGE charges ~625ns per DMA)
            s0 = (m - 1) % 6
            dma_eng.dma_start(
                _split_pk(yT[(m - 1) * P : (m + 1) * P, jc]),
                ysb4[:, s0 : s0 + 2, :],
            )

    def emit_divisions(jd, pairs, tail=False):
        # normalize head pair i2 (heads i2, i2+4) of chunk jd.  Pairs 0-2
        # read recA (ready one block early); pair 3 reads recB.
        if jd < 0:
            return
        jcd = slice(jd * 512, (jd + 1) * 512)
        for i2 in pairs:
            for u in (0, 1):
                h = i2 + 4 * u
                rec = recB if i2 == 3 else recA
                rsel = selB_sb if i2 == 3 else sel_sb[0:6, :]
                rps = scr_tile()
                nc.tensor.matmul(
                    rps[0:64, :],
                    rsel[:, h * 64 : (h + 1) * 64],
                    rec,
                    start=True,
                    stop=True,
                )
                if u == 0:
                    nc.vector.tensor_tensor(
                        OT_sb[0:64, i2, jcd], Oraw[:, h, :], rps[0:64, :], op=mult
                    )
                else:
                    otmp = tmp.tile([64, 512], BF, tag="otmp", bufs=2, name="otmp")
                    nc.vector.tensor_tensor(
                        otmp, Oraw[:, h, :], rps[0:64, :], op=mult
                    )
                    dma_eng = nc.scalar if tail else nc.sync
                    dma_eng.dma_start(OT_sb[64:128, i2, jcd], otmp)

    for j in range(4):
        jc = slice(j * 512, (j + 1) * 512)
        den_dram = dram.tile([8, 512], F32, tag="dend", bufs=2, name="dend")
        for i in range(4):
            pavA = pav_tile()
            pavB = pav_tile()

            def av_step(k):
                for u, pav in ((0, pavA), (1, pavB)):
                    nc.tensor.matmul(
                        pav[0:65, :],
                        V_sb[:, k, u * 65 : (u + 1) * 65],
                        E_sb[:, k % 5, u, :],
                        start=(k == 0),
                        stop=(k == 15),
                    )

            for k in range(16):
                S_t = S_tile()
                ks = slice(k * P, (k + 1) * P)
                nc.tensor.matmul(
                    S_t[:, 0:512], Kr_sb[0:64, ks], Qr_sb[0:64, i, jc],
                    start=True, stop=True,
                )
                nc.tensor.matmul(
                    S_t[:, 512:1024], Kr_sb[64:128, ks], Qr_sb[64:128, i, jc],
                    start=True, stop=True,
                )
                nc.scalar.activation(
                    E_sb[:, k % 5, :, :],
                    S_t.rearrange("p (u c) -> p u c", c=512),
                    Exp,
                    scale=0.125,
                )
                # fill PE slack: V tiles + remaining K chunks during (0,0),
                # both lagged to match the x DMA stream; exactly one deferred
                # Q chunk per block; previous chunk's o-proj during j>0;
                # current chunk's pair 0-2 divisions at i==3
                if j == 0 and i == 0:
                    if k >= 1:
                        v_chunk(k - 1)
                    if k in (1, 5, 9):
                        expand_rope((k + 3) // 4)
                    if k in (3, 7, 11):
                        kproj_chunk((k + 1) // 4)
                if k == (14 if (j, i) == (0, 0) else 1):
                    fi, jq = (i + 1, j) if i < 3 else (0, j + 1)
                    if jq < 4 and not (fi == 0 and jq == 0):
                        q_chunk(fi, jq)
                if k >= 2:
                    av_step(k - 2)
                oproj_ks = (8, 9, 13, 15) if i == 0 else (7, 9, 13, 15)
                if j > 0 and k in oproj_ks:
                    if not (j == 3 and k >= 11):
                        oproj_tile(j - 1, i * 4 + oproj_ks.index(k))
                if i == 0 and k == 4:
                    emit_divisions(j - 1, [3])  # leftover pair of prev chunk
                if i == 3 and k in (5, 10, 14):
                    emit_divisions(j, [(k == 10) + 2 * (k == 14)])
            if (j, i) == (0, 0):
                v_chunk(15)
            av_step(14)
            av_step(15)
            # denominator copies + DMAs first (they head the recip chain the
            # next divisions wait on), Oraw copies after (consumed later)
            dens = []
            for u, pav in ((0, pavA), (1, pavB)):
                den1 = tmp.tile([P, 512], F32, tag="den1", bufs=2, name="den1")
                nc.vector.tensor_copy(den1[64:65, :], pav[64:65, :])
                dens.append(den1)
            for u, den1 in enumerate(dens):
                nc.sync.dma_start(den_dram[2 * i + u : 2 * i + u + 1, :],
                                  den1[64:65, :])
            for u, pav in ((0, pavA), (1, pavB)):
                nc.vector.tensor_copy(Oraw[:, i + 4 * u, :], pav[0:64, :])
            if i == 2:
                # pairs 0-2 denominators complete: reciprocal one block early
                nc.sync.dma_start(denA, den_dram[0:6, :])
                nc.vector.reciprocal_approx_accurate(recA, denA, scratch=rscr)
        nc.sync.dma_start(denB, den_dram[6:8, :])
        nc.vector.reciprocal_approx_accurate(recB, denB, scratch=rscr[0:2, :])

    # tail: chunk 2's eight deferred o-proj tiles overlap the pair-3 recip
    # chain, then pair 3 of chunk 3 divides, then the last chunk's o-proj
    # (k2 order puts pair 3 last, so only m=0 briefly waits)
    for m in (2, 3, 6, 7, 10, 11, 14, 15):
        oproj_tile(2, m, tail=True)
    emit_divisions(3, [3], tail=True)
    for m in range(16):
        oproj_tile(3, m, tail=True, single_dma=(m >= 14))

    for p in (pp, dram, tmp, wts, big, cst):
        p.release()


def _build():
    if "nc" in _CACHE:
        return _CACHE["nc"]
    nc = bacc.Bacc("TRN2", target_bir_lowering=False, debug=False, num_devices=NCORES)
    xT = nc.dram_tensor("xT", [HIDDEN, SEQ], BF, kind="ExternalInput").ap()
    wqT = nc.dram_tensor("wqT", [HIDDEN, FH], BF, kind="ExternalInput").ap()
    wkT = nc.dram_tensor("wkT", [HIDDEN, 128], BF, kind="ExternalInput").ap()
    wvT = nc.dram_tensor("wvT", [HIDDEN, 128], BF, kind="ExternalInput").ap()
    woT = nc.dram_tensor("woT", [FH, HIDDEN], BF, kind="ExternalInput").ap()
    cosD = nc.dram_tensor("cosE", [32, SEQ], BF, kind="ExternalInput").ap()
    sinD = nc.dram_tensor("sinE", [32, SEQ], BF, kind="ExternalInput").ap()
    xpD = nc.dram_tensor("xpand", [32, 128], BF, kind="ExternalInput").ap()
    rotD = nc.dram_tensor("rotP", [128, 128], BF, kind="ExternalInput").ap()
    selD = nc.dram_tensor("sel8", [8, 512], F32, kind="ExternalInput").ap()
    yT = nc.dram_tensor("yT", [HIDDEN, SEQ], F32, kind="ExternalOutput").ap()
    with tile.TileContext(nc) as tc:
        _emit(tc, nc, (xT, wqT, wkT, wvT, woT, cosD, sinD, rotD, selD, xpD, yT))
    nc.compile()
    _CACHE["nc"] = nc
    return nc


def _in_maps(hidden_states, wq, wk, wv, wo):
    cosE, sinE, rotP, sel8, xpand = _host_constants()
    maps = []
    for c in range(NCORES):
        b, g = c // 4, c % 4
        feat = np.concatenate(
            [np.arange(64) + 64 * (8 * g + hl) for hl in PERM]
        )
        maps.append(
            {
                "xT": np.ascontiguousarray(hidden_states[b].T).astype(BF16NP),
                "wqT": np.ascontiguousarray(wq[feat, :].T).astype(BF16NP),
                "wkT": np.ascontiguousarray(
                    wk[128 * g : 128 * (g + 1), :].T
                ).astype(BF16NP),
                "wvT": np.ascontiguousarray(
                    wv[128 * g : 128 * (g + 1), :].T
                ).astype(BF16NP),
                "woT": np.ascontiguousarray(wo[:, feat].T).astype(BF16NP),
                "cosE": cosE,
                "sinE": sinE,
                "xpand": xpand,
                "rotP": rotP,
                "sel8": sel8,
            }
        )
    return maps


def kernel(hidden_states, wq, wk, wv, wo):
    nc = _build()
    maps = _in_maps(
        np.asarray(hidden_states, dtype=np.float32),
        np.asarray(wq, dtype=np.float32),
        np.asarray(wk, dtype=np.float32),
        np.asarray(wv, dtype=np.float32),
        np.asarray(wo, dtype=np.float32),
    )
    res = bass_utils.run_bass_kernel_spmd(nc, maps, list(range(NCORES))).results
    y = np.zeros((BATCH, SEQ, HIDDEN), dtype=np.float64)
    for c in range(NCORES):
        y[c // 4] += res[c]["yT"].T.astype(np.float64)
    return y.astype(np.float32)
